# revision 1
# baseline (speedup 1.0000x reference)
"""Multi-head attention (B=4, L=2048, D=1024, H=16) on 8 NeuronCores.

Sharding: core c handles batch b=c//2 and query rows [1024*(c%2), +1024).
The per-core input x is the batch's [2048, 1024] activations ROTATED so the
core's own query rows are rows 0..1023 (softmax over keys is permutation
invariant, so rotating keys+values together is exact). No collectives needed.

Per-core pipeline (all matmuls in float32r = full-speed ~tf32 precision):
  A)  transpose x -> XT [k, s] (PE transpose); QT = Wq^T@XT[:, :1024],
      KT = Wk^T@XT (SBUF resident); V = XT^T@Wv staged to DRAM with a
      fused ones-column per head (for the softmax denominator).
  B1) per head pair: scores^T tile [s,l] = KT_h^T @ QT_h (contraction d=64,
      row-group paired across the 2 heads); exp via ScalarE (scale=1/8
      folded); PV accumulate [V_h|1]^T @ exp(S^T) -> [65, l] PSUM where row
      64 = softmax denominator; normalize rows 0..63 by broadcasted
      reciprocal.
  C)  y^T = Wo^T @ OT (+bo fused), PE-transpose back to [l, dout], DMA out.
"""

import numpy as np

import sys

for _p in ("/opt/trn_rl_repo", "/opt/pypackages"):
    if _p not in sys.path:
        sys.path.append(_p)

from contextlib import ExitStack

import concourse.bass as bass
import concourse.mybir as mybir
import concourse.tile as tile
from concourse import bacc
from concourse.bass_utils import run_bass_kernel_spmd
from concourse.masks import make_identity

B, L, D, H = 4, 2048, 1024, 16
HD = D // H  # 64
LQ = 1024  # query rows per core
N_CORES = 8
F32 = mybir.dt.float32
F32R = mybir.dt.float32r
AF = mybir.ActivationFunctionType

P = 128
KT_TILES = D // P  # 8 k tiles
ST_TILES = L // P  # 16 s tiles
DT_TILES = D // P  # 8 d tiles
LH = 512  # l half width
SCALE = 1.0 / float(np.sqrt(HD))
PIPELINE = True
COMBINED_EXP = True
B1_LHALF = True
B1_XPAIR = True


def _load_bias(nc, pool, dram, name):
    """[1024] dram vector -> [128, 8] sbuf tile; column t = b[128t:128t+128]."""
    t = pool.tile([P, DT_TILES], F32, name=name)
    nc.gpsimd.dma_start(t[:], dram.rearrange("(t p) -> p t", p=P))
    return t


def build_nc(repeat=1, stop_after=None):
    nc = bacc.Bacc(None)

    x_d = nc.declare_dram_parameter("x", [L, D], F32, isOutput=False)
    wq_d = nc.declare_dram_parameter("wq", [D, D], F32, isOutput=False)
    wk_d = nc.declare_dram_parameter("wk", [D, D], F32, isOutput=False)
    wv_d = nc.declare_dram_parameter("wv", [D, D], F32, isOutput=False)
    wo_d = nc.declare_dram_parameter("wo", [D, D], F32, isOutput=False)
    bq_d = nc.declare_dram_parameter("bq", [D], F32, isOutput=False)
    bk_d = nc.declare_dram_parameter("bk", [D], F32, isOutput=False)
    bv_d = nc.declare_dram_parameter("bv", [D], F32, isOutput=False)
    bo_d = nc.declare_dram_parameter("bo", [D], F32, isOutput=False)
    y_d = nc.declare_dram_parameter("y", [LQ, D], F32, isOutput=True)

    # V staged in DRAM, already augmented with a ones column per head:
    # [s_tile, partition(s), head, 65] where col 64 of each head slot is 1.0
    v_dram = nc.dram_tensor("v_stage", [ST_TILES, P, H, HD + 1], F32R)

    with tile.TileContext(nc) as tc, ExitStack() as ctx:
      for _rep in range(repeat):
       with ExitStack() as rctx:
        singles = rctx.enter_context(tc.tile_pool(name="singles", bufs=1))
        ident32 = singles.tile([P, P], F32, name="ident32")
        make_identity(nc, ident32[:])
        ident = singles.tile([P, P], F32R, name="ident")
        nc.vector.tensor_copy(ident[:], ident32[:])
        bq_sb = _load_bias(nc, singles, bq_d, "bq")
        bk_sb = _load_bias(nc, singles, bk_d, "bk")
        bv_sb = _load_bias(nc, singles, bv_d, "bv")
        bo_sb = _load_bias(nc, singles, bo_d, "bo")

        # big resident slabs
        qt_pool = rctx.enter_context(tc.tile_pool(name="qt", bufs=1))
        kt_pool = rctx.enter_context(tc.tile_pool(name="kt", bufs=1))
        qt = qt_pool.tile([P, DT_TILES, LQ], F32R, name="qt")  # [d%128, dtile, l]
        kt = kt_pool.tile([P, DT_TILES, L], F32R, name="kt")  # [d%128, dtile, s]

        # ---------------- Phase A: transpose + projections ----------------
        with (
            tc.tile_pool(name="xt", bufs=1) as xt_pool,
            tc.tile_pool(name="wpool", bufs=2) as wpool,
            tc.tile_pool(name="vb", bufs=3) as vb_pool,
            tc.tile_pool(name="wv", bufs=1) as wv_pool,
            tc.tile_pool(name="ps_proj", bufs=4, space="PSUM") as ps_proj,
        ):
            xt = xt_pool.tile([P, KT_TILES, L], F32R, name="xt")  # [k%128, ktile, s]

            # transpose x into xt (PE transpose of 128x128 blocks)
            with (
                tc.tile_pool(name="xpool", bufs=3) as xpool,
                tc.tile_pool(name="ps_tr", bufs=3, space="PSUM") as ps_tr,
            ):
                for li in range(ST_TILES):
                    # plain HWDGE fp32 load; fp32->fp32r cast happens for free
                    # in the transpose-evict copy below (4 transposes batched
                    # into one PSUM bank -> single DVE eviction)
                    x_sb = xpool.tile([P, D], F32, name="x_sb")
                    nc.sync.dma_start(x_sb[:], x_d[li * P : (li + 1) * P, :])
                    for kg in range(KT_TILES // 4):
                        pt4 = ps_tr.tile([P, 4, P], F32, name="pt4")
                        for b in range(4):
                            ki = 4 * kg + b
                            nc.tensor.transpose(
                                pt4[:, b, :], x_sb[:, ki * P : (ki + 1) * P], ident32[:]
                            )
                        nc.vector.tensor_copy(
                            xt[:, 4 * kg : 4 * kg + 4, li * P : (li + 1) * P], pt4[:]
                        )

            # QT[d, l] = sum_k Wq[k, d-tile]^T @ XT[k, l]   (+bq fused)
            # KT[d, s] = sum_k Wk[k, d-tile]^T @ XT[k, s]   (+bk fused)
            # W column block per d-tile: [128(k%128), ktile, 128(d)]
            for w_d, b_sb, out_sb, ncols in (
                (wq_d, bq_sb, qt, LQ),
                (wk_d, bk_sb, kt, L),
            ):
                for dt_i in range(DT_TILES):
                    w_col = wpool.tile([P, KT_TILES, P], F32R, name="w_col")
                    nc.gpsimd.dma_start(
                        w_col[:],
                        w_d[:, dt_i * P : (dt_i + 1) * P].rearrange(
                            "(t p) n -> p t n", p=P
                        ),
                    )
                    for ci in range(ncols // LH):
                        ps = ps_proj.tile([P, LH], F32, name="ps_proj")
                        for ki in range(KT_TILES):
                            nc.tensor.matmul(
                                ps[:],
                                w_col[:, ki, :],
                                xt[:, ki, ci * LH : (ci + 1) * LH],
                                start=(ki == 0),
                                stop=(ki == KT_TILES - 1),
                            )
                        nc.scalar.activation(
                            out_sb[:, dt_i, ci * LH : (ci + 1) * LH],
                            ps[:],
                            AF.Identity,
                            bias=b_sb[:, dt_i : dt_i + 1],
                        )

            # V[s, d] = sum_k XT[k, s-tile]^T @ Wv[k, d] staged to DRAM
            # bounce buffer interleaves the per-head ones column.
            for dc in range(2):  # 512-wide chunks = 8 heads each
                wv_half = wv_pool.tile([P, KT_TILES, LH], F32R, name="wv_half")
                nc.gpsimd.dma_start(
                    wv_half[:],
                    wv_d[:, dc * LH : (dc + 1) * LH].rearrange("(t p) n -> p t n", p=P),
                )
                for st in range(ST_TILES):
                    ps = ps_proj.tile([P, LH], F32, name="ps_proj")
                    for ki in range(KT_TILES):
                        nc.tensor.matmul(
                            ps[:],
                            xt[:, ki, st * P : (st + 1) * P],
                            wv_half[:, ki, :],
                            start=(ki == 0),
                            stop=(ki == KT_TILES - 1),
                        )
                    vb = vb_pool.tile([P, 8, HD + 1], F32R, name="vb")
                    nc.vector.memset(vb[:, :, HD : HD + 1].bitcast(F32), 1.0)
                    nc.vector.tensor_copy(vb[:, :, 0:HD], ps[:])
                    nc.sync.dma_start(v_dram[st, :, dc * 8 : (dc + 1) * 8, :], vb[:])

        if stop_after == "a":
            for i in range(KT_TILES):
                nc.sync.dma_start(y_d[i * P : (i + 1) * P, :], qt[:, i, :].bitcast(F32))
            continue

        # ---------------- Phase B1: attention per head pair ----------------
        ot_pool = rctx.enter_context(tc.tile_pool(name="ot", bufs=1))
        ot = ot_pool.tile([P, DT_TILES, LQ], F32R, name="ot")  # [din%128, dintile, l]

        with (
            tc.tile_pool(name="vaug", bufs=2) as vaug_pool,
            tc.tile_pool(name="et", bufs=(5 if B1_LHALF else 3 if COMBINED_EXP else 6)) as et_pool,
            tc.tile_pool(name="otmp", bufs=3) as otmp_pool,
            tc.tile_pool(name="rr", bufs=2) as rr_pool,
            tc.tile_pool(name="rb", bufs=2) as rb_pool,
            tc.tile_pool(name="ps_s", bufs=(2 if B1_LHALF else 1 if COMBINED_EXP else 2), space="PSUM") as ps_s_pool,
            tc.tile_pool(name="ps_o", bufs=2, space="PSUM") as ps_o_pool,
        ):
            if B1_XPAIR:
                # flat unit pipeline across pair boundaries: the lookahead-1
                # scores/exp never drains at a pair boundary
                pair_vaug = {}
                pair_pso = {}

                def ensure_vaug(p):
                    if p not in pair_vaug:
                        v = vaug_pool.tile(
                            [P, ST_TILES, 2 * (HD + 1)], F32R, name="vaug"
                        )
                        nc.sync.dma_start(
                            v[:],
                            v_dram[:, :, 2 * p : 2 * p + 2, :].rearrange(
                                "s p h c -> p s (h c)"
                            ),
                        )
                        pair_vaug[p] = v

                def scores_g(p, st, lh):
                    ps_s = ps_s_pool.tile([P, 2, LH], F32, name="ps_s")
                    for sub in range(2):
                        nc.tensor.matmul(
                            ps_s[:, sub, :],
                            kt[sub * HD : (sub + 1) * HD, p, st * P : (st + 1) * P],
                            qt[sub * HD : (sub + 1) * HD, p, lh * LH : (lh + 1) * LH],
                            start=True,
                            stop=True,
                        )
                    e2 = et_pool.tile([P, 2, LH], F32R, name="et")
                    nc.scalar.activation(e2[:], ps_s[:], AF.Exp, scale=SCALE)
                    return e2

                def pv_g(p, st, lh, e2):
                    if p not in pair_pso:
                        pair_pso[p] = [
                            ps_o_pool.tile([HD + 1, LQ], F32, name="ps_o")
                            for _ in range(2)
                        ]
                    po = pair_pso[p]
                    for sub in range(2):
                        nc.tensor.matmul(
                            po[sub][:, lh * LH : (lh + 1) * LH],
                            pair_vaug[p][:, st, sub * (HD + 1) : (sub + 1) * (HD + 1)],
                            e2[:, sub, :],
                            start=(st == 0),
                            stop=(st == ST_TILES - 1),
                        )

                def epilogue(p):
                    po = pair_pso.pop(p)
                    pair_vaug.pop(p)
                    for sub in range(2):
                        o_tmp = otmp_pool.tile([HD + 1, LQ], F32, name="o_tmp")
                        nc.vector.tensor_copy(o_tmp[:], po[sub][:])
                        r_row = rr_pool.tile([1, LQ], F32, name="r_row")
                        nc.vector.reciprocal(r_row[:], o_tmp[HD : HD + 1, :])
                        r_bc = rb_pool.tile([HD, LQ], F32, name="r_bc")
                        nc.gpsimd.partition_broadcast(r_bc[:], r_row[:])
                        dst = ot[sub * HD : (sub + 1) * HD, p, :]
                        nc.vector.tensor_mul(dst, o_tmp[0:HD, :], r_bc[:])
                        nc.vector.tensor_scalar_add(
                            dst, dst, bv_sb[sub * HD : (sub + 1) * HD, p : p + 1]
                        )

                all_units = [
                    (p, st, lh)
                    for p in range(H // 2)
                    for st in range(ST_TILES)
                    for lh in range(2)
                ]
                prev = None
                for u in all_units:
                    ensure_vaug(u[0])
                    e2 = scores_g(*u)
                    if prev is not None:
                        pv_g(*prev[0], prev[1])
                        if prev[0][1] == ST_TILES - 1 and prev[0][2] == 1:
                            epilogue(prev[0][0])
                    prev = (u, e2)
                pv_g(*prev[0], prev[1])
                epilogue(prev[0][0])

            for pair in ([] if B1_XPAIR else range(H // 2)):
                vaug = vaug_pool.tile([P, ST_TILES, 2 * (HD + 1)], F32R, name="vaug")
                nc.sync.dma_start(
                    vaug[:],
                    v_dram[:, :, 2 * pair : 2 * pair + 2, :].rearrange(
                        "s p h c -> p s (h c)"
                    ),
                )
                ps_o = [
                    ps_o_pool.tile([HD + 1, LQ], F32, name="ps_o") for _ in range(2)
                ]

                def scores_exp_lh(st, lh):
                    # 2-bank scores tile (both subs, one l-half): restores
                    # ps_s double-buffering within the 8-bank PSUM budget
                    ps_s = ps_s_pool.tile([P, 2, LH], F32, name="ps_s")
                    for sub in range(2):
                        nc.tensor.matmul(
                            ps_s[:, sub, :],
                            kt[sub * HD : (sub + 1) * HD, pair, st * P : (st + 1) * P],
                            qt[sub * HD : (sub + 1) * HD, pair, lh * LH : (lh + 1) * LH],
                            start=True,
                            stop=True,
                        )
                    e2 = et_pool.tile([P, 2, LH], F32R, name="et")
                    nc.scalar.activation(e2[:], ps_s[:], AF.Exp, scale=SCALE)
                    return e2

                def pv_lh(st, lh, e2):
                    for sub in range(2):
                        nc.tensor.matmul(
                            ps_o[sub][:, lh * LH : (lh + 1) * LH],
                            vaug[:, st, sub * (HD + 1) : (sub + 1) * (HD + 1)],
                            e2[:, sub, :],
                            start=(st == 0),
                            stop=(st == ST_TILES - 1),
                        )

                def scores_exp(st):
                    if COMBINED_EXP:
                        # both heads' scores into one 4-bank PSUM tile so a
                        # SINGLE [128, 2048] ACTIVATE covers them (halves the
                        # per-op ScalarE overhead)
                        ps_s = ps_s_pool.tile([P, 2, LQ], F32, name="ps_s")
                        for sub in range(2):
                            for lh in range(2):
                                nc.tensor.matmul(
                                    ps_s[:, sub, lh * LH : (lh + 1) * LH],
                                    kt[sub * HD : (sub + 1) * HD, pair, st * P : (st + 1) * P],
                                    qt[sub * HD : (sub + 1) * HD, pair, lh * LH : (lh + 1) * LH],
                                    start=True,
                                    stop=True,
                                )
                        e2 = et_pool.tile([P, 2, LQ], F32R, name="et")
                        nc.scalar.activation(e2[:], ps_s[:], AF.Exp, scale=SCALE)
                        return [e2[:, 0, :], e2[:, 1, :]]
                    et = [None, None]
                    for sub in range(2):
                        ps_s = ps_s_pool.tile([P, LQ], F32, name="ps_s")
                        for lh in range(2):
                            nc.tensor.matmul(
                                ps_s[:, lh * LH : (lh + 1) * LH],
                                kt[sub * HD : (sub + 1) * HD, pair, st * P : (st + 1) * P],
                                qt[sub * HD : (sub + 1) * HD, pair, lh * LH : (lh + 1) * LH],
                                start=True,
                                stop=True,
                            )
                        e = et_pool.tile([P, LQ], F32R, name="et")
                        nc.scalar.activation(e[:], ps_s[:], AF.Exp, scale=SCALE)
                        et[sub] = e
                    return et

                def pv(st, et):
                    for sub in range(2):
                        for lh in range(2):
                            nc.tensor.matmul(
                                ps_o[sub][:, lh * LH : (lh + 1) * LH],
                                vaug[:, st, sub * (HD + 1) : (sub + 1) * (HD + 1)],
                                et[sub][:, lh * LH : (lh + 1) * LH],
                                start=(st == 0),
                                stop=(st == ST_TILES - 1),
                            )

                if B1_LHALF:
                    units = [(st, lh) for st in range(ST_TILES) for lh in range(2)]
                    e_cur = scores_exp_lh(*units[0])
                    for i, u in enumerate(units):
                        e_next = (
                            scores_exp_lh(*units[i + 1]) if i + 1 < len(units) else None
                        )
                        pv_lh(*u, e_cur)
                        e_cur = e_next
                elif PIPELINE:
                    # software pipeline: scores(st+1) emitted before pv(st) so
                    # PE has independent work while ACT computes exp(st)
                    et_cur = scores_exp(0)
                    for st in range(ST_TILES):
                        et_next = scores_exp(st + 1) if st + 1 < ST_TILES else None
                        pv(st, et_cur)
                        et_cur = et_next
                else:
                    for st in range(ST_TILES):
                        pv(st, scores_exp(st))
                # evict O+denominator to SBUF immediately (frees the PSUM
                # bank for the next pair), then normalize rows 0..63 by the
                # broadcasted reciprocal of row 64, write into ot slab (+bv).
                for sub in range(2):
                    o_tmp = otmp_pool.tile([HD + 1, LQ], F32, name="o_tmp")
                    nc.vector.tensor_copy(o_tmp[:], ps_o[sub][:])
                    r_row = rr_pool.tile([1, LQ], F32, name="r_row")
                    nc.vector.reciprocal(r_row[:], o_tmp[HD : HD + 1, :])
                    r_bc = rb_pool.tile([HD, LQ], F32, name="r_bc")
                    nc.gpsimd.partition_broadcast(r_bc[:], r_row[:])
                    dst = ot[sub * HD : (sub + 1) * HD, pair, :]
                    nc.vector.tensor_mul(dst, o_tmp[0:HD, :], r_bc[:])
                    nc.vector.tensor_scalar_add(
                        dst, dst, bv_sb[sub * HD : (sub + 1) * HD, pair : pair + 1]
                    )

        if stop_after == "ab":
            for i in range(KT_TILES):
                nc.sync.dma_start(y_d[i * P : (i + 1) * P, :], ot[:, i, :].bitcast(F32))
            continue

        # ---------------- Phase C: output projection + transpose ----------------
        with (
            tc.tile_pool(name="wo", bufs=2) as wo_pool,
            tc.tile_pool(name="gt", bufs=2) as gt_pool,
            tc.tile_pool(name="ysl", bufs=1) as y_pool,
            tc.tile_pool(name="ps_g", bufs=2, space="PSUM") as ps_g_pool,
            tc.tile_pool(name="ps_t", bufs=3, space="PSUM") as ps_t_pool,
        ):
            y_sb = y_pool.tile([P, KT_TILES, D], F32, name="y_sb")  # [l%128, ltile, dout]
            for j in range(DT_TILES):  # dout tiles
                wo_sb = wo_pool.tile([P, KT_TILES, P], F32R, name="wo_sb")
                nc.gpsimd.dma_start(
                    wo_sb[:],
                    wo_d[:, j * P : (j + 1) * P].rearrange("(t p) n -> p t n", p=P),
                )
                gt_s = gt_pool.tile([P, LQ], F32R, name="gt_s")
                for lh in range(2):
                    ps_g = ps_g_pool.tile([P, LH], F32, name="ps_g")
                    for ki in range(KT_TILES):
                        nc.tensor.matmul(
                            ps_g[:],
                            wo_sb[:, ki, :],
                            ot[:, ki, lh * LH : (lh + 1) * LH],
                            start=(ki == 0),
                            stop=(ki == KT_TILES - 1),
                        )
                    nc.scalar.activation(
                        gt_s[:, lh * LH : (lh + 1) * LH],
                        ps_g[:],
                        AF.Identity,
                        bias=bo_sb[:, j : j + 1],
                    )
                for a in range(KT_TILES // 4):  # l tiles, batched 4-per-bank
                    pt4 = ps_t_pool.tile([P, 4, P], F32R, name="pt4_out")
                    for b in range(4):
                        i = 4 * a + b
                        nc.tensor.transpose(
                            pt4[:, b, :], gt_s[:, i * P : (i + 1) * P], ident[:]
                        )
                    nc.vector.tensor_copy(
                        y_sb[:, 4 * a : 4 * a + 4, j * P : (j + 1) * P], pt4[:]
                    )
            for i in range(KT_TILES):
                nc.sync.dma_start(y_d[i * P : (i + 1) * P, :], y_sb[:, i, :])

    nc.finalize()
    return nc


_NC_CACHE = None


def kernel(**inputs):
    global _NC_CACHE
    if _NC_CACHE is None:
        _NC_CACHE = build_nc()
    nc = _NC_CACHE

    q = np.ascontiguousarray(np.asarray(inputs["q"], dtype=np.float32))
    w = {k: np.ascontiguousarray(np.asarray(inputs[k], dtype=np.float32))
         for k in ("Wq", "Wk", "Wv", "Wo", "bq", "bk", "bv", "bo")}

    in_maps = []
    for c in range(N_CORES):
        b, half = c // 2, c % 2
        lo = LQ * half
        x_rot = np.concatenate([q[b, lo:], q[b, :lo]], axis=0)
        in_maps.append({
            "x": np.ascontiguousarray(x_rot),
            "wq": w["Wq"], "wk": w["Wk"], "wv": w["Wv"], "wo": w["Wo"],
            "bq": w["bq"], "bk": w["bk"], "bv": w["bv"], "bo": w["bo"],
        })

    res = run_bass_kernel_spmd(nc, in_maps, core_ids=list(range(N_CORES)))

    out = np.empty((B, L, D), dtype=np.float32)
    for c in range(N_CORES):
        b, half = c // 2, c % 2
        lo = LQ * half
        out[b, lo : lo + LQ, :] = res.results[c]["y"]
    return out



# revision 8
# speedup vs baseline: 1.3786x; 1.3786x over previous
"""Multi-head attention (B=4, L=2048, D=1024, H=16) on 8 NeuronCores.

Sharding: core c handles batch b=c//2 and query rows [1024*(c%2), +1024).
The per-core input x is the batch's [2048, 1024] activations ROTATED so the
core's own query rows are rows 0..1023 (softmax over keys is permutation
invariant, so rotating keys+values together is exact). No collectives needed.

All matmul operands are bf16 (same PE rate as f32r here, but half the
SBUF/DMA traffic); accumulation is fp32 PSUM. V (augmented with a per-head
ones column whose PV row becomes the softmax denominator) stays
SBUF-resident. x^T is produced by XBAR transpose-DMAs, not the PE.

The kernel is ONE flat instruction stream: after a short prologue
(x transpose-DMAs + pair-0 projections), the per-pair attention units
(scores -> exp -> PV, lookahead-1) run with the projection chunks for LATER
pairs interleaved between units, so the PE always has independent work
while the Activation engine computes exp (~1.0us/unit vs 0.85us of PE work
per unit). All PSUM evictions ride on DVE/Pool, keeping ACT exp-only. The
output projection for l-half 0 interleaves into the last pair's units.

PSUM budget (8 banks): scores [128,2,512]x2 = 4, PV accum [65,512]x2 = 2,
projections [128,512]x2 = 2; the projection pool is closed before the
output-projection pool (2 banks) opens.
"""

import numpy as np

import sys

for _p in ("/opt/trn_rl_repo", "/opt/pypackages"):
    if _p not in sys.path:
        sys.path.append(_p)

from contextlib import ExitStack

import concourse.bass as bass
import concourse.mybir as mybir
import concourse.tile as tile
from concourse import bacc
from concourse.bass_utils import run_bass_kernel_spmd
from concourse.masks import make_identity

B, L, D, H = 4, 2048, 1024, 16
HD = D // H  # 64
LQ = 1024  # query rows per core
N_CORES = 8
F32 = mybir.dt.float32
BF16 = mybir.dt.bfloat16
AF = mybir.ActivationFunctionType

P = 128
KT_TILES = D // P  # 8 k tiles
ST_TILES = L // P  # 16 s tiles
DT_TILES = D // P  # 8 d tiles = 8 head pairs
LH = 512  # l half width
N_PAIR = H // 2  # 8
SCALE = 1.0 / float(np.sqrt(HD))


def _load_bias(nc, pool, dram, name):
    """[1024] dram vector -> [128, 8] sbuf tile; column t = b[128t:128t+128]."""
    t = pool.tile([P, DT_TILES], F32, name=name)
    nc.gpsimd.dma_start(t[:], dram.rearrange("(t p) -> p t", p=P))
    return t


def build_nc(repeat=1):
    nc = bacc.Bacc(None)

    x_d = nc.declare_dram_parameter("x", [L, D], BF16, isOutput=False)
    wq_d = nc.declare_dram_parameter("wq", [D, D], BF16, isOutput=False)
    wk_d = nc.declare_dram_parameter("wk", [D, D], BF16, isOutput=False)
    wv_d = nc.declare_dram_parameter("wv", [D, D], BF16, isOutput=False)
    wo_d = nc.declare_dram_parameter("wo", [D, D], BF16, isOutput=False)
    bq_d = nc.declare_dram_parameter("bq", [D], F32, isOutput=False)
    bk_d = nc.declare_dram_parameter("bk", [D], F32, isOutput=False)
    bv_d = nc.declare_dram_parameter("bv", [D], F32, isOutput=False)
    bo_d = nc.declare_dram_parameter("bo", [D], F32, isOutput=False)
    y_d = nc.declare_dram_parameter("y", [LQ, D], F32, isOutput=True)

    with tile.TileContext(nc) as tc, ExitStack() as ctx:
      for _rep in range(repeat):
       with ExitStack() as rctx:
        singles = rctx.enter_context(tc.tile_pool(name="singles", bufs=1))
        ident32 = singles.tile([P, P], F32, name="ident32")
        make_identity(nc, ident32[:])
        ident = singles.tile([P, P], BF16, name="ident")
        nc.vector.tensor_copy(ident[:], ident32[:])
        bq_sb = _load_bias(nc, singles, bq_d, "bq")
        bk_sb = _load_bias(nc, singles, bk_d, "bk")
        bv_sb = _load_bias(nc, singles, bv_d, "bv")
        bo_sb = _load_bias(nc, singles, bo_d, "bo")

        # big resident slabs (bf16)
        slab = rctx.enter_context(tc.tile_pool(name="slab", bufs=1))
        qt = slab.tile([P, DT_TILES, LQ], BF16, name="qt")  # [d%128, pair, l]
        kt = slab.tile([P, DT_TILES, L], BF16, name="kt")  # [d%128, pair, s]
        ot = slab.tile([P, DT_TILES, LQ], BF16, name="ot")  # [din%128, dt, l]
        xt = slab.tile([P, KT_TILES, L], BF16, name="xt")  # [k%128, ktile, s]
        # V augmented: [s%128, st, head, 64 vals | 1.0]
        vaug = slab.tile([P, ST_TILES, H, HD + 1], BF16, name="vaug")
        nc.vector.memset(vaug[:, :, :, HD : HD + 1], 1.0)

        wo_sb = slab.tile([P, KT_TILES, D], BF16, name="wo_sb")
        gt_sb = slab.tile([P, DT_TILES, LQ], BF16, name="gt_sb")

        wpool = rctx.enter_context(tc.tile_pool(name="wpool", bufs=6))
        et_pool = rctx.enter_context(tc.tile_pool(name="et", bufs=3))
        otmp_pool = rctx.enter_context(tc.tile_pool(name="otmp", bufs=2))
        rr_pool = rctx.enter_context(tc.tile_pool(name="rr", bufs=2))
        rb_pool = rctx.enter_context(tc.tile_pool(name="rb", bufs=2))

        # ---- weight-tile loads, issued ~one pair-group ahead of use ----
        w_tiles = {}
        wload_queue = [(k, dt) for dt in range(DT_TILES) for k in ("q", "k", "v")]
        w_drams = {"q": wq_d, "k": wk_d, "v": wv_d}

        def issue_next_wload():
            if not wload_queue:
                return
            kind, dt = wload_queue.pop(0)
            t = wpool.tile([P, KT_TILES, P], BF16, name="w_col")
            nc.gpsimd.dma_start(
                t[:],
                w_drams[kind][:, dt * P : (dt + 1) * P].rearrange(
                    "(t p) n -> p t n", p=P
                ),
            )
            w_tiles[(kind, dt)] = t

        # The projection pool lives on the RIGHT side of PSUM so it can be
        # closed mid-stream (the left-side ps_s/ps_o stack stays LIFO) and
        # replaced by the output-projection pool in the same 2 banks.
        proj_stack = ExitStack()
        ps_proj = proj_stack.enter_context(
            tc.tile_pool(name="ps_proj", bufs=2, space="PSUM", side="right")
        )
        ps_g_holder = {}

        def ensure_ps_g():
            if "g" not in ps_g_holder:
                proj_stack.close()  # frees the 2 projection banks
                ps_g_holder["g"] = rctx.enter_context(
                    tc.tile_pool(name="ps_g", bufs=2, space="PSUM", side="right")
                )
            return ps_g_holder["g"]

        # =================== task emitters ===================
        def proj_qk_chunk(kind, dt, ci):
            """One 512-wide column chunk of the Q or K projection for
            d-tile dt; bias added during the DVE eviction."""
            if ci == 0:
                issue_next_wload()
            w_col = w_tiles[(kind, dt)]
            out_sb, b_sb = (qt, bq_sb) if kind == "q" else (kt, bk_sb)
            ps = ps_proj.tile([P, LH], F32, name="ps_proj")
            for ki in range(KT_TILES):
                nc.tensor.matmul(
                    ps[:],
                    w_col[:, ki, :],
                    xt[:, ki, ci * LH : (ci + 1) * LH],
                    start=(ki == 0),
                    stop=(ki == KT_TILES - 1),
                )
            nc.vector.tensor_scalar_add(
                out_sb[:, dt, ci * LH : (ci + 1) * LH], ps[:], b_sb[:, dt : dt + 1]
            )

        def proj_v_chunk(pair, g):
            """V projection for head pair `pair`, s-tiles 4g..4g+3, staged
            into vaug (cols 0..63 per head; col 64 stays the memset 1.0)."""
            if g == 0:
                issue_next_wload()
            w_col = w_tiles[("v", pair)]
            ps = ps_proj.tile([P, LH], F32, name="ps_proj")
            for sti in range(4):
                st = 4 * g + sti
                for ki in range(KT_TILES):
                    nc.tensor.matmul(
                        ps[:, sti * P : (sti + 1) * P],
                        xt[:, ki, st * P : (st + 1) * P],
                        w_col[:, ki, :],
                        start=(ki == 0),
                        stop=(ki == KT_TILES - 1),
                    )
            nc.vector.tensor_copy(
                vaug[:, 4 * g : 4 * g + 4, 2 * pair : 2 * pair + 2, 0:HD],
                ps[:].rearrange("p (s h d) -> p s h d", s=4, h=2),
            )

        def load_wo():
            nc.gpsimd.dma_start(wo_sb[:], wo_d.rearrange("(t p) n -> p t n", p=P))

        def c_proj_chunk(lt, jg):
            """Output projection y^T chunk: 4 dout-tiles for l-tile lt,
            bias bo added in the Pool eviction."""
            ps_g_pool = ensure_ps_g()
            ps_g = ps_g_pool.tile([P, 4, P], F32, name="ps_g")
            for jj in range(4):
                j = 4 * jg + jj
                for ki in range(KT_TILES):
                    nc.tensor.matmul(
                        ps_g[:, jj, :],
                        wo_sb[:, ki, j * P : (j + 1) * P],
                        ot[:, ki, lt * P : (lt + 1) * P],
                        start=(ki == 0),
                        stop=(ki == KT_TILES - 1),
                    )
                nc.vector.tensor_scalar_add(
                    gt_sb[:, j, lt * P : (lt + 1) * P],
                    ps_g[:, jj, :],
                    bo_sb[:, j : j + 1],
                )

        # =================== B1 unit emitters ===================
        def scores_unit(ps_s_pool, p, lh, st):
            ps_s = ps_s_pool.tile([P, 2, LH], F32, name="ps_s")
            for sub in range(2):
                nc.tensor.matmul(
                    ps_s[:, sub, :],
                    kt[sub * HD : (sub + 1) * HD, p, st * P : (st + 1) * P],
                    qt[sub * HD : (sub + 1) * HD, p, lh * LH : (lh + 1) * LH],
                    start=True,
                    stop=True,
                )
            e2 = et_pool.tile([P, 2, LH], BF16, name="et")
            nc.scalar.activation(e2[:], ps_s[:], AF.Exp, scale=SCALE)
            return e2

        def pv_unit(ps_o, p, lh, st, e2):
            for sub in range(2):
                nc.tensor.matmul(
                    ps_o[sub][:],
                    vaug[:, st, 2 * p + sub, :],
                    e2[:, sub, :],
                    start=(st == 0),
                    stop=(st == ST_TILES - 1),
                )

        def epilogue(ps_o, p, lh):
            """Drain the [65, 512] PV accumulators: row 64 is the softmax
            denominator; normalize rows 0..63, add bv, write ot (bf16)."""
            for sub in range(2):
                o_tmp = otmp_pool.tile([HD + 1, LH], F32, name="o_tmp")
                nc.vector.tensor_copy(o_tmp[:], ps_o[sub][:])
                r_row = rr_pool.tile([1, LH], F32, name="r_row")
                nc.vector.reciprocal(r_row[:], o_tmp[HD : HD + 1, :])
                r_bc = rb_pool.tile([HD, LH], F32, name="r_bc")
                nc.gpsimd.partition_broadcast(r_bc[:], r_row[:])
                dst = ot[sub * HD : (sub + 1) * HD, p, lh * LH : (lh + 1) * LH]
                nc.vector.tensor_mul(dst, o_tmp[0:HD, :], r_bc[:])
                nc.vector.tensor_scalar_add(
                    dst, dst, bv_sb[sub * HD : (sub + 1) * HD, p : p + 1]
                )

        # ============================================================
        # Prologue: XBAR transpose-DMAs bring x^T into xt while pair-0
        # projections are emitted behind them.
        # ============================================================
        issue_next_wload()  # q0
        issue_next_wload()  # k0
        issue_next_wload()  # v0
        # XBAR transpose must land in a contiguous tile (a sliced slab
        # destination produces wrong data on hardware); bounce + DVE copy.
        with tc.tile_pool(name="xtb", bufs=3) as xtb_pool:
            for sh in range(2):  # s halves, so pair-0 work can start early
                for t in range(KT_TILES):
                    eng = (nc.sync, nc.scalar)[t % 2]
                    bt = xtb_pool.tile([P, LQ], BF16, name="xtb")
                    eng.dma_start_transpose(
                        bt[:],
                        x_d[sh * LQ : (sh + 1) * LQ, t * P : (t + 1) * P],
                    )
                    nc.vector.tensor_copy(xt[:, t, sh * LQ : (sh + 1) * LQ], bt[:])
        for kind, a, b in (
            ("q", 0, 0), ("q", 0, 1), ("k", 0, 0), ("k", 0, 1),
            ("v", 0, 0), ("v", 0, 1), ("k", 0, 2), ("k", 0, 3),
            ("v", 0, 2), ("v", 0, 3),
        ):
            if kind == "v":
                proj_v_chunk(a, b)
            else:
                proj_qk_chunk(kind, a, b)

        # ============================================================
        # Main flat pipeline: B1 units with interleaved A/C chunks.
        # ============================================================
        units = [
            (p, lh, st)
            for p in range(N_PAIR)
            for lh in range(2)
            for st in range(ST_TILES)
        ]
        u_of = {u: i for i, u in enumerate(units)}

        a_tasks = []  # (emit_fn, earliest_unit, deadline_unit)
        for p2 in range(1, N_PAIR):
            dl = u_of[(p2, 0, 0)]
            for ci in range(2):
                a_tasks.append((lambda k=p2, c=ci: proj_qk_chunk("q", k, c), 0, dl))
            for ci in range(4):
                a_tasks.append((lambda k=p2, c=ci: proj_qk_chunk("k", k, c), 0, dl))
            for g in range(4):
                a_tasks.append((lambda k=p2, g2=g: proj_v_chunk(k, g2), 0, dl))
        a_tasks.append((load_wo, u_of[(6, 0, 0)], u_of[(7, 0, 0)]))
        for lt in range(4):  # phase C, l-half 0, rides inside pair-7/lh=1
            for jg in range(2):
                a_tasks.append(
                    (
                        lambda l2=lt, j2=jg: c_proj_chunk(l2, j2),
                        u_of[(7, 1, 1)] + 2 * (2 * lt + jg),
                        10**9,
                    )
                )

        bstack = ExitStack()
        ps_s_pool = bstack.enter_context(
            tc.tile_pool(name="ps_s", bufs=2, space="PSUM")
        )
        ps_o_pool = bstack.enter_context(
            tc.tile_pool(name="ps_o", bufs=2, space="PSUM")
        )

        task_state = {"i": 0}

        def drain(i):
            while task_state["i"] < len(a_tasks):
                fn, earliest, deadline = a_tasks[task_state["i"]]
                if deadline <= i or (earliest <= i and 3 * task_state["i"] <= i + 9):
                    fn()
                    task_state["i"] += 1
                else:
                    break

        pair_pso = {}

        def ensure_pso(p, lh):
            if (p, lh) not in pair_pso:
                pair_pso[(p, lh)] = [
                    ps_o_pool.tile([HD + 1, LH], F32, name="ps_o") for _ in range(2)
                ]
            return pair_pso[(p, lh)]

        prev = None
        for i, u in enumerate(units):
            drain(i)
            e2 = scores_unit(ps_s_pool, *u)
            if prev is not None:
                pu, pe = prev
                pv_unit(ensure_pso(pu[0], pu[1]), *pu, pe)
                if pu[2] == ST_TILES - 1:
                    epilogue(pair_pso.pop((pu[0], pu[1])), pu[0], pu[1])
            prev = (u, e2)
        pu, pe = prev
        pv_unit(ensure_pso(pu[0], pu[1]), *pu, pe)
        epilogue(pair_pso.pop((pu[0], pu[1])), pu[0], pu[1])
        while task_state["i"] < len(a_tasks):  # stragglers (C lh0 chunks)
            a_tasks[task_state["i"]][0]()
            task_state["i"] += 1

        bstack.close()  # frees scores + PV PSUM banks

        # ---------------- phase C tail ----------------
        with (
            tc.tile_pool(name="ps_ct", bufs=3, space="PSUM") as ps_ct,
            tc.tile_pool(name="yrow", bufs=2) as y_pool,
        ):

            def emit_y(lt):
                """PE-transpose gt (y^T, bf16) back to [l, dout], evict to
                f32 and DMA out one 128-row slice of y."""
                y_row = y_pool.tile([P, D], F32, name="y_row")
                for a in range(2):
                    pt4 = ps_ct.tile([P, 4, P], BF16, name="pt4_out")
                    for b2 in range(4):
                        j = 4 * a + b2
                        nc.tensor.transpose(
                            pt4[:, b2, :],
                            gt_sb[:, j, lt * P : (lt + 1) * P],
                            ident[:],
                        )
                    nc.vector.tensor_copy(y_row[:, a * LH : (a + 1) * LH], pt4[:])
                nc.sync.dma_start(y_d[lt * P : (lt + 1) * P, :], y_row[:])

            for lt in range(4, KT_TILES):  # l-half-1 projections + overlap
                for jg in range(2):
                    c_proj_chunk(lt, jg)
                emit_y(lt - 4)
            for lt in range(4, KT_TILES):
                emit_y(lt)

    nc.finalize()
    return nc


_NC_CACHE = None


def kernel(**inputs):
    global _NC_CACHE
    if _NC_CACHE is None:
        _NC_CACHE = build_nc()
    nc = _NC_CACHE

    import ml_dtypes

    bf16 = ml_dtypes.bfloat16
    q = np.ascontiguousarray(np.asarray(inputs["q"], dtype=np.float32))
    wb = {}
    for k in ("Wq", "Wk", "Wv", "Wo"):
        wb[k] = np.ascontiguousarray(np.asarray(inputs[k]).astype(bf16))
    for k in ("bq", "bk", "bv", "bo"):
        wb[k] = np.ascontiguousarray(np.asarray(inputs[k], dtype=np.float32))

    in_maps = []
    for c in range(N_CORES):
        b, half = c // 2, c % 2
        lo = LQ * half
        x_rot = np.concatenate([q[b, lo:], q[b, :lo]], axis=0).astype(bf16)
        in_maps.append({
            "x": np.ascontiguousarray(x_rot),
            "wq": wb["Wq"], "wk": wb["Wk"], "wv": wb["Wv"], "wo": wb["Wo"],
            "bq": wb["bq"], "bk": wb["bk"], "bv": wb["bv"], "bo": wb["bo"],
        })

    res = run_bass_kernel_spmd(nc, in_maps, core_ids=list(range(N_CORES)))

    out = np.empty((B, L, D), dtype=np.float32)
    for c in range(N_CORES):
        b, half = c // 2, c % 2
        lo = LQ * half
        out[b, lo : lo + LQ, :] = res.results[c]["y"]
    return out


# revision 15
# speedup vs baseline: 1.3868x; 1.0060x over previous
"""Multi-head attention (B=4, L=2048, D=1024, H=16) on 8 NeuronCores.

Sharding: core c handles batch b=c//2 and query rows [1024*(c%2), +1024).
The per-core input x is the batch's [2048, 1024] activations ROTATED so the
core's own query rows are rows 0..1023 (softmax over keys is permutation
invariant, so rotating keys+values together is exact). No collectives needed.

All matmul operands are bf16 (same PE rate as f32r here, but half the
SBUF/DMA traffic); accumulation is fp32 PSUM. V (augmented with a per-head
ones column whose PV row becomes the softmax denominator) stays
SBUF-resident. x^T is produced by XBAR transpose-DMAs, not the PE.

The kernel is ONE flat instruction stream: after a short prologue
(x transpose-DMAs + pair-0 projections), the per-pair attention units
(scores -> exp -> PV, lookahead-1) run with the projection chunks for LATER
pairs interleaved between units, so the PE always has independent work
while the Activation engine computes exp (~1.0us/unit vs 0.85us of PE work
per unit). All PSUM evictions ride on DVE/Pool, keeping ACT exp-only. The
output projection for l-half 0 interleaves into the last pair's units.

PSUM budget (8 banks): scores [128,2,512]x2 = 4, PV accum [65,512]x2 = 2,
projections [128,512]x2 = 2; the projection pool is closed before the
output-projection pool (2 banks) opens.
"""

import numpy as np

import sys

for _p in ("/opt/trn_rl_repo", "/opt/pypackages"):
    if _p not in sys.path:
        sys.path.append(_p)

from contextlib import ExitStack

import concourse.bass as bass
import concourse.mybir as mybir
import concourse.tile as tile
from concourse import bacc
from concourse.bass_utils import run_bass_kernel_spmd
from concourse.masks import make_identity

B, L, D, H = 4, 2048, 1024, 16
HD = D // H  # 64
LQ = 1024  # query rows per core
N_CORES = 8
F32 = mybir.dt.float32
BF16 = mybir.dt.bfloat16
AF = mybir.ActivationFunctionType

P = 128
KT_TILES = D // P  # 8 k tiles
ST_TILES = L // P  # 16 s tiles
DT_TILES = D // P  # 8 d tiles = 8 head pairs
LH = 512  # l half width
N_PAIR = H // 2  # 8
SCALE = 1.0 / float(np.sqrt(HD))


def _load_bias(nc, pool, dram, name):
    """[1024] dram vector -> [128, 8] sbuf tile; column t = b[128t:128t+128]."""
    t = pool.tile([P, DT_TILES], F32, name=name)
    nc.gpsimd.dma_start(t[:], dram.rearrange("(t p) -> p t", p=P))
    return t


def build_nc(repeat=1):
    nc = bacc.Bacc(None)

    x_d = nc.declare_dram_parameter("x", [L, D], BF16, isOutput=False)
    wq_d = nc.declare_dram_parameter("wq", [D, D], BF16, isOutput=False)
    wk_d = nc.declare_dram_parameter("wk", [D, D], BF16, isOutput=False)
    wv_d = nc.declare_dram_parameter("wv", [D, D], BF16, isOutput=False)
    wo_d = nc.declare_dram_parameter("wo", [D, D], BF16, isOutput=False)
    bq_d = nc.declare_dram_parameter("bq", [D], F32, isOutput=False)
    bk_d = nc.declare_dram_parameter("bk", [D], F32, isOutput=False)
    bv_d = nc.declare_dram_parameter("bv", [D], F32, isOutput=False)
    bo_d = nc.declare_dram_parameter("bo", [D], F32, isOutput=False)
    y_d = nc.declare_dram_parameter("y", [LQ, D], F32, isOutput=True)

    with tile.TileContext(nc) as tc, ExitStack() as ctx:
      for _rep in range(repeat):
       with ExitStack() as rctx:
        singles = rctx.enter_context(tc.tile_pool(name="singles", bufs=1))
        ident32 = singles.tile([P, P], F32, name="ident32")
        make_identity(nc, ident32[:])
        ident = singles.tile([P, P], BF16, name="ident")
        nc.vector.tensor_copy(ident[:], ident32[:])
        bq_sb = _load_bias(nc, singles, bq_d, "bq")
        bk_sb = _load_bias(nc, singles, bk_d, "bk")
        bv_sb = _load_bias(nc, singles, bv_d, "bv")
        bo_sb = _load_bias(nc, singles, bo_d, "bo")

        # big resident slabs (bf16)
        slab = rctx.enter_context(tc.tile_pool(name="slab", bufs=1))
        qt = slab.tile([P, DT_TILES, LQ], BF16, name="qt")  # [d%128, pair, l]
        kt = slab.tile([P, DT_TILES, L], BF16, name="kt")  # [d%128, pair, s]
        ot = slab.tile([P, DT_TILES, LQ], BF16, name="ot")  # [din%128, dt, l]
        xt = slab.tile([P, KT_TILES, L], BF16, name="xt")  # [k%128, ktile, s]
        # V augmented: [s%128, st, head, 64 vals | 1.0]
        vaug = slab.tile([P, ST_TILES, H, HD + 1], BF16, name="vaug")
        nc.vector.memset(vaug[:, :, :, HD : HD + 1], 1.0)

        wo_sb = slab.tile([P, KT_TILES, D], BF16, name="wo_sb")
        gt_sb = slab.tile([P, DT_TILES, LQ], BF16, name="gt_sb")

        wpool = rctx.enter_context(tc.tile_pool(name="wpool", bufs=6))
        et_pool = rctx.enter_context(tc.tile_pool(name="et", bufs=3))
        otmp_pool = rctx.enter_context(tc.tile_pool(name="otmp", bufs=2))
        rr_pool = rctx.enter_context(tc.tile_pool(name="rr", bufs=2))
        rb_pool = rctx.enter_context(tc.tile_pool(name="rb", bufs=2))

        # ---- weight-tile loads, issued ~one pair-group ahead of use ----
        w_tiles = {}
        wload_queue = [(k, dt) for dt in range(DT_TILES) for k in ("q", "k", "v")]
        w_drams = {"q": wq_d, "k": wk_d, "v": wv_d}

        def issue_next_wload():
            if not wload_queue:
                return
            kind, dt = wload_queue.pop(0)
            t = wpool.tile([P, KT_TILES, P], BF16, name="w_col")
            nc.gpsimd.dma_start(
                t[:],
                w_drams[kind][:, dt * P : (dt + 1) * P].rearrange(
                    "(t p) n -> p t n", p=P
                ),
            )
            w_tiles[(kind, dt)] = t

        # The projection pool lives on the RIGHT side of PSUM so it can be
        # closed mid-stream (the left-side ps_s/ps_o stack stays LIFO) and
        # replaced by the output-projection pool in the same 2 banks.
        proj_stack = ExitStack()
        ps_proj = proj_stack.enter_context(
            tc.tile_pool(name="ps_proj", bufs=2, space="PSUM", side="right")
        )
        proj_pools = {"p": ps_proj}  # swapped to a wider pool in the prologue
        ps_g_holder = {}

        def ensure_ps_g():
            if "g" not in ps_g_holder:
                proj_stack.close()  # frees the 2 projection banks
                ps_g_holder["g"] = rctx.enter_context(
                    tc.tile_pool(name="ps_g", bufs=2, space="PSUM", side="right")
                )
            return ps_g_holder["g"]

        # =================== task emitters ===================
        def proj_qk_chunk(kind, dt, ci):
            """One 512-wide column chunk of the Q or K projection for
            d-tile dt; bias added during the DVE eviction."""
            if ci == 0:
                issue_next_wload()
            w_col = w_tiles[(kind, dt)]
            out_sb, b_sb = (qt, bq_sb) if kind == "q" else (kt, bk_sb)
            ps = proj_pools["p"].tile([P, LH], F32, name="ps_proj")
            for ki in range(KT_TILES):
                nc.tensor.matmul(
                    ps[:],
                    w_col[:, ki, :],
                    xt[:, ki, ci * LH : (ci + 1) * LH],
                    start=(ki == 0),
                    stop=(ki == KT_TILES - 1),
                )
            nc.vector.tensor_scalar_add(
                out_sb[:, dt, ci * LH : (ci + 1) * LH], ps[:], b_sb[:, dt : dt + 1]
            )

        def proj_v_chunk(pair, g):
            """V projection for head pair `pair`, s-tiles 4g..4g+3, staged
            into vaug (cols 0..63 per head; col 64 stays the memset 1.0)."""
            if g == 0:
                issue_next_wload()
            w_col = w_tiles[("v", pair)]
            ps = proj_pools["p"].tile([P, LH], F32, name="ps_proj")
            for sti in range(4):
                st = 4 * g + sti
                for ki in range(KT_TILES):
                    nc.tensor.matmul(
                        ps[:, sti * P : (sti + 1) * P],
                        xt[:, ki, st * P : (st + 1) * P],
                        w_col[:, ki, :],
                        start=(ki == 0),
                        stop=(ki == KT_TILES - 1),
                    )
            nc.vector.tensor_copy(
                vaug[:, 4 * g : 4 * g + 4, 2 * pair : 2 * pair + 2, 0:HD],
                ps[:].rearrange("p (s h d) -> p s h d", s=4, h=2),
            )

        def load_wo():
            nc.gpsimd.dma_start(wo_sb[:], wo_d.rearrange("(t p) n -> p t n", p=P))

        def c_proj_chunk(lt, jg, on_act=False):
            """Output projection y^T chunk: 4 dout-tiles for l-tile lt.
            Evict on DVE while attention still runs; on the (then idle)
            ACT engine in the tail."""
            ps_g_pool = ensure_ps_g()
            ps_g = ps_g_pool.tile([P, 4, P], F32, name="ps_g")
            for jj in range(4):
                j = 4 * jg + jj
                for ki in range(KT_TILES):
                    nc.tensor.matmul(
                        ps_g[:, jj, :],
                        wo_sb[:, ki, j * P : (j + 1) * P],
                        ot[:, ki, lt * P : (lt + 1) * P],
                        start=(ki == 0),
                        stop=(ki == KT_TILES - 1),
                    )
                if on_act:
                    nc.scalar.activation(
                        gt_sb[:, j, lt * P : (lt + 1) * P],
                        ps_g[:, jj, :],
                        AF.Identity,
                        bias=bo_sb[:, j : j + 1],
                    )
                else:
                    nc.vector.tensor_scalar_add(
                        gt_sb[:, j, lt * P : (lt + 1) * P],
                        ps_g[:, jj, :],
                        bo_sb[:, j : j + 1],
                    )

        # =================== B1 unit emitters ===================
        def scores_unit(ps_s_pool, p, lh, st):
            ps_s = ps_s_pool.tile([P, 2, LH], F32, name="ps_s")
            for sub in range(2):
                nc.tensor.matmul(
                    ps_s[:, sub, :],
                    kt[sub * HD : (sub + 1) * HD, p, st * P : (st + 1) * P],
                    qt[sub * HD : (sub + 1) * HD, p, lh * LH : (lh + 1) * LH],
                    start=True,
                    stop=True,
                )
            e2 = et_pool.tile([P, 2, LH], BF16, name="et")
            nc.scalar.activation(e2[:], ps_s[:], AF.Exp, scale=SCALE)
            return e2

        def pv_unit(ps_o, p, lh, st, e2):
            for sub in range(2):
                nc.tensor.matmul(
                    ps_o[sub][:],
                    vaug[:, st, 2 * p + sub, :],
                    e2[:, sub, :],
                    start=(st == 0),
                    stop=(st == ST_TILES - 1),
                )

        def epilogue(ps_o, p, lh):
            """Drain the [65, 512] PV accumulators: row 64 is the softmax
            denominator; normalize rows 0..63, add bv, write ot (bf16)."""
            for sub in range(2):
                o_tmp = otmp_pool.tile([HD + 1, LH], F32, name="o_tmp")
                nc.vector.tensor_copy(o_tmp[:], ps_o[sub][:])
                r_row = rr_pool.tile([1, LH], F32, name="r_row")
                nc.vector.reciprocal(r_row[:], o_tmp[HD : HD + 1, :])
                r_bc = rb_pool.tile([HD, LH], F32, name="r_bc")
                nc.gpsimd.partition_broadcast(r_bc[:], r_row[:])
                dst = ot[sub * HD : (sub + 1) * HD, p, lh * LH : (lh + 1) * LH]
                nc.vector.tensor_mul(dst, o_tmp[0:HD, :], r_bc[:])
                nc.vector.tensor_scalar_add(
                    dst, dst, bv_sb[sub * HD : (sub + 1) * HD, p : p + 1]
                )

        # ============================================================
        # Prologue: XBAR transpose-DMAs bring x^T into xt while pair-0
        # projections are emitted behind them.
        # ============================================================
        issue_next_wload()  # q0
        issue_next_wload()  # k0
        issue_next_wload()  # v0
        # XBAR transpose must land in a contiguous tile (a sliced slab
        # destination produces wrong data on hardware); bounce + DVE copy.
        # s-half 0 lands first; its dependent pair-0 chunks are emitted
        # before the s-half-1 bounce copies so the DVE queue can't delay
        # the first projection evictions. A dedicated prologue PSUM pool
        # (right side, closed before B1) widens projection double-buffering.
        with (
            tc.tile_pool(name="xtb", bufs=3) as xtb_pool,
            tc.tile_pool(name="ps_prol", bufs=3, space="PSUM", side="right") as ps_prol,
        ):
            # bounce copies ride the (otherwise idle) Pool engine so the DVE
            # queue is free for the projection evictions from the start
            for sh in range(2):
                for t in range(KT_TILES):
                    eng = (nc.sync, nc.scalar)[t % 2]
                    bt = xtb_pool.tile([P, LQ], BF16, name="xtb")
                    eng.dma_start_transpose(
                        bt[:],
                        x_d[sh * LQ : (sh + 1) * LQ, t * P : (t + 1) * P],
                    )
                    nc.gpsimd.tensor_copy(
                        xt[:, t, sh * LQ : (sh + 1) * LQ], bt[:]
                    )
            saved_pool, proj_pools["p"] = proj_pools["p"], ps_prol
            for kind, a, b in (
                ("q", 0, 0), ("q", 0, 1), ("k", 0, 0), ("k", 0, 1),
                ("v", 0, 0), ("v", 0, 1), ("k", 0, 2), ("k", 0, 3),
                ("v", 0, 2), ("v", 0, 3),
            ):
                if kind == "v":
                    proj_v_chunk(a, b)
                else:
                    proj_qk_chunk(kind, a, b)
            proj_pools["p"] = saved_pool

        # ============================================================
        # Main flat pipeline: B1 units with interleaved A/C chunks.
        # ============================================================
        units = [
            (p, lh, st)
            for p in range(N_PAIR)
            for lh in range(2)
            for st in range(ST_TILES)
        ]
        u_of = {u: i for i, u in enumerate(units)}

        a_tasks = []  # (emit_fn, earliest_unit, deadline_unit)
        for p2 in range(1, N_PAIR):
            dl = u_of[(p2, 0, 0)]
            for ci in range(2):
                a_tasks.append((lambda k=p2, c=ci: proj_qk_chunk("q", k, c), 0, dl))
            for ci in range(4):
                a_tasks.append((lambda k=p2, c=ci: proj_qk_chunk("k", k, c), 0, dl))
            for g in range(4):
                a_tasks.append((lambda k=p2, g2=g: proj_v_chunk(k, g2), 0, dl))
        a_tasks.append((load_wo, u_of[(6, 0, 0)], u_of[(7, 0, 0)]))
        for lt in range(4):  # phase C, l-half 0, rides inside pair-7/lh=1
            for jg in range(2):
                a_tasks.append(
                    (
                        lambda l2=lt, j2=jg: c_proj_chunk(l2, j2),
                        u_of[(7, 1, 1)] + 2 * (2 * lt + jg),
                        10**9,
                    )
                )

        bstack = ExitStack()
        ps_s_pool = bstack.enter_context(
            tc.tile_pool(name="ps_s", bufs=2, space="PSUM")
        )
        ps_o_pool = bstack.enter_context(
            tc.tile_pool(name="ps_o", bufs=2, space="PSUM")
        )

        task_state = {"i": 0}

        def drain(i):
            while task_state["i"] < len(a_tasks):
                fn, earliest, deadline = a_tasks[task_state["i"]]
                if deadline <= i or (earliest <= i and 3 * task_state["i"] <= i + 9):
                    fn()
                    task_state["i"] += 1
                else:
                    break

        pair_pso = {}

        def ensure_pso(p, lh):
            if (p, lh) not in pair_pso:
                pair_pso[(p, lh)] = [
                    ps_o_pool.tile([HD + 1, LH], F32, name="ps_o") for _ in range(2)
                ]
            return pair_pso[(p, lh)]

        prev = None
        for i, u in enumerate(units):
            drain(i)
            e2 = scores_unit(ps_s_pool, *u)
            if prev is not None:
                pu, pe = prev
                pv_unit(ensure_pso(pu[0], pu[1]), *pu, pe)
                if pu[2] == ST_TILES - 1:
                    epilogue(pair_pso.pop((pu[0], pu[1])), pu[0], pu[1])
            prev = (u, e2)
        pu, pe = prev
        pv_unit(ensure_pso(pu[0], pu[1]), *pu, pe)
        epilogue(pair_pso.pop((pu[0], pu[1])), pu[0], pu[1])
        while task_state["i"] < len(a_tasks):  # stragglers (C lh0 chunks)
            a_tasks[task_state["i"]][0]()
            task_state["i"] += 1

        bstack.close()  # frees scores + PV PSUM banks

        # ---------------- phase C tail ----------------
        with (
            tc.tile_pool(name="ps_ct", bufs=3, space="PSUM") as ps_ct,
            tc.tile_pool(name="yrow", bufs=2) as y_pool,
        ):

            def emit_y(lt):
                """PE-transpose gt (y^T, bf16) back to [l, dout], evict to
                f32 and DMA out one 128-row slice of y."""
                y_row = y_pool.tile([P, D], F32, name="y_row")
                for a in range(2):
                    pt4 = ps_ct.tile([P, 4, P], BF16, name="pt4_out")
                    for b2 in range(4):
                        j = 4 * a + b2
                        nc.tensor.transpose(
                            pt4[:, b2, :],
                            gt_sb[:, j, lt * P : (lt + 1) * P],
                            ident[:],
                        )
                    nc.vector.tensor_copy(y_row[:, a * LH : (a + 1) * LH], pt4[:])
                nc.sync.dma_start(y_d[lt * P : (lt + 1) * P, :], y_row[:])

            for lt in range(4, KT_TILES):  # l-half-1 projections + overlap
                for jg in range(2):
                    c_proj_chunk(lt, jg, on_act=True)
                emit_y(lt - 4)
            for lt in range(4, KT_TILES):
                emit_y(lt)

    nc.finalize()
    return nc


_NC_CACHE = None


def kernel(**inputs):
    global _NC_CACHE
    if _NC_CACHE is None:
        _NC_CACHE = build_nc()
    nc = _NC_CACHE

    import ml_dtypes

    bf16 = ml_dtypes.bfloat16
    q = np.ascontiguousarray(np.asarray(inputs["q"], dtype=np.float32))
    wb = {}
    for k in ("Wq", "Wk", "Wv", "Wo"):
        wb[k] = np.ascontiguousarray(np.asarray(inputs[k]).astype(bf16))
    for k in ("bq", "bk", "bv", "bo"):
        wb[k] = np.ascontiguousarray(np.asarray(inputs[k], dtype=np.float32))

    in_maps = []
    for c in range(N_CORES):
        b, half = c // 2, c % 2
        lo = LQ * half
        x_rot = np.concatenate([q[b, lo:], q[b, :lo]], axis=0).astype(bf16)
        in_maps.append({
            "x": np.ascontiguousarray(x_rot),
            "wq": wb["Wq"], "wk": wb["Wk"], "wv": wb["Wv"], "wo": wb["Wo"],
            "bq": wb["bq"], "bk": wb["bk"], "bv": wb["bv"], "bo": wb["bo"],
        })

    res = run_bass_kernel_spmd(nc, in_maps, core_ids=list(range(N_CORES)))

    out = np.empty((B, L, D), dtype=np.float32)
    for c in range(N_CORES):
        b, half = c // 2, c % 2
        lo = LQ * half
        out[b, lo : lo + LQ, :] = res.results[c]["y"]
    return out
